# revision 6
# baseline (speedup 1.0000x reference)
"""Trainium2 Bass kernel for nn_Attention_48687749268214.

Self-attention with pair-bias: LN(x) -> qkv -> q/k LN -> heads,
bias = einsum('bijc,hc->bhij', LN(pair), w_bias), softmax(qk/8+bias) @ v -> proj.

Sharding: sequence-shard the i axis across 8 cores (64 query rows each).
Each core gets its pair slice pair[i0:i0+64] (j-rolled so that the core's own
query rows sit at local rows 0:63 in its rolled copy of x), computes its own
64 output rows with no collectives; host concatenates.

Key math fold: bias = LN(pair) @ (pn_g * w_bias).T + pn_b @ w_bias.T, with the
LN applied in the natural [ij, c] layout (per-partition mean/rstd scalars),
then PE-transposed tiles feed a [12 x 512] matmul per i-row panel.
"""

import sys

sys.path.insert(0, "/opt/trn_rl_repo")

from contextlib import ExitStack

import ml_dtypes
import numpy as np

import concourse.bass as bass
import concourse.tile as tile
from concourse import bacc, mybir
from concourse.bass_utils import run_bass_kernel_spmd
from concourse.masks import make_identity

F32 = mybir.dt.float32
BF16 = mybir.dt.bfloat16
AF = mybir.ActivationFunctionType
OP = mybir.AluOpType

C = 768
H = 12
HD = 64
N = 512
NCORES = 8
IB = N // NCORES  # 64 i rows per core
NIJ = IB * N  # 32768 pair rows per core
EPS = 1e-5
RC = 1.0 / C

bf = ml_dtypes.bfloat16


def _build():
    nc = bacc.Bacc(
        "TRN2", target_bir_lowering=False, debug=False, num_devices=NCORES
    )

    pair_d = nc.dram_tensor("pair_s", [NIJ, C], F32, kind="ExternalInput").ap()
    x_d = nc.dram_tensor("x_s", [N, C], F32, kind="ExternalInput").ap()
    wqkvt_d = nc.dram_tensor("wqkvt", [C, 3 * C], BF16, kind="ExternalInput").ap()
    bqkv_d = nc.dram_tensor("bqkv", [1, 3 * C], BF16, kind="ExternalInput").ap()
    wprojt_d = nc.dram_tensor("wprojt", [C, C], BF16, kind="ExternalInput").ap()
    bproj_d = nc.dram_tensor("bproj", [1, C], BF16, kind="ExternalInput").ap()
    wg_d = nc.dram_tensor("wg", [C, H], BF16, kind="ExternalInput").ap()
    sbrow_d = nc.dram_tensor("sbrow", [1, H], BF16, kind="ExternalInput").ap()
    reps_d = nc.dram_tensor("lnreps", [6, C], BF16, kind="ExternalInput").ap()
    out_d = nc.dram_tensor("out", [IB, C], F32, kind="ExternalOutput").ap()
    biasd = nc.dram_tensor("bias_scratch", [H, NIJ], BF16).ap()

    with tile.TileContext(nc) as tc, ExitStack() as ctx:
        sing = ctx.enter_context(tc.tile_pool(name="sing", bufs=1))
        pairp = ctx.enter_context(tc.tile_pool(name="pairp", bufs=2))
        dumpp = ctx.enter_context(tc.tile_pool(name="dumpp", bufs=2))
        statp = ctx.enter_context(tc.tile_pool(name="statp", bufs=3))
        ptp = ctx.enter_context(tc.tile_pool(name="ptp", bufs=2))
        stagep = ctx.enter_context(tc.tile_pool(name="stagep", bufs=3))
        attnp = ctx.enter_context(tc.tile_pool(name="attnp", bufs=2))
        ps_t = ctx.enter_context(tc.tile_pool(name="ps_t", bufs=6, space="PSUM"))
        ps_mm = ctx.enter_context(tc.tile_pool(name="ps_mm", bufs=2, space="PSUM"))

        # ---- singles / weights ----
        id128 = sing.tile([128, 128], BF16)
        make_identity(nc, id128)
        ones_col = sing.tile([1, 128], BF16)
        nc.vector.memset(ones_col, 1.0)
        ones_row = sing.tile([1, N], BF16)
        nc.vector.memset(ones_row, 1.0)
        epsT = sing.tile([128, 1], F32)
        nc.vector.memset(epsT, EPS)

        wqkvt = sing.tile([128, 6, 3 * C], BF16)
        nc.sync.dma_start(out=wqkvt, in_=wqkvt_d.rearrange("(k p) o -> p k o", p=128))
        wprojt = sing.tile([128, 6, C], BF16)
        nc.sync.dma_start(out=wprojt, in_=wprojt_d.rearrange("(k p) o -> p k o", p=128))
        wg = sing.tile([128, 6, H], BF16)
        nc.sync.dma_start(out=wg, in_=wg_d.rearrange("(k p) o -> p k o", p=128))
        sbrow = sing.tile([1, H], BF16)
        nc.sync.dma_start(out=sbrow, in_=sbrow_d)
        bqkv = sing.tile([1, 3 * C], BF16)
        nc.sync.dma_start(out=bqkv, in_=bqkv_d)
        bproj = sing.tile([1, C], BF16)
        nc.sync.dma_start(out=bproj, in_=bproj_d)

        # replicated LN params: rows = ln_g, ln_b, qln_g, qln_b, kln_g, kln_b
        reps = sing.tile([128, 6, C], BF16)
        for rI in range(6):
            nc.gpsimd.dma_start(
                out=reps[:, rI, :],
                in_=bass.AP(
                    tensor=reps_d.tensor, offset=rI * C, ap=[[0, 128], [1, C]]
                ),
            )

        def ln_stats(src_ap, s1_ap, s2_ap, dA, dB):
            """accumulate sum (DVE) and sumsq (ACT) of a [p, C] tile."""
            nc.vector.tensor_scalar(
                out=dA, in0=src_ap, scalar1=1.0, scalar2=0.0, op0=OP.mult,
                op1=OP.add, accum_out=s1_ap,
            )
            nc.scalar.activation(out=dB, in_=src_ap, func=AF.Square, accum_out=s2_ap)

        def ln_mu_r(s1, s2, mu, r, tmp, w):
            """mu = s1/C ; r = rsqrt(s2/C - mu^2 + eps), all [p, w] f32."""
            nc.vector.tensor_scalar(
                out=mu[:, 0:w], in0=s1[:, 0:w], scalar1=RC, scalar2=None, op0=OP.mult
            )
            nc.vector.tensor_tensor(
                out=tmp[:, 0:w], in0=mu[:, 0:w], in1=mu[:, 0:w], op=OP.mult
            )
            nc.vector.scalar_tensor_tensor(
                out=tmp[:, 0:w], in0=s2[:, 0:w], scalar=RC, in1=tmp[:, 0:w],
                op0=OP.mult, op1=OP.subtract,
            )
            nc.scalar.activation(
                out=tmp[:, 0:w], in_=tmp[:, 0:w], func=AF.Ln, bias=epsT
            )
            nc.scalar.activation(out=r[:, 0:w], in_=tmp[:, 0:w], func=AF.Exp, scale=-0.5)

        # ================= x path =================
        x_sb = sing.tile([128, 4, C], F32)
        nc.sync.dma_start(out=x_sb, in_=x_d.rearrange("(t p) c -> p t c", p=128))
        xn = sing.tile([128, 4, C], BF16)
        dxA = dumpp.tile([128, C], BF16, tag="dump")
        dxB = dumpp.tile([128, C], BF16, tag="dump2")
        s1x = statp.tile([128, 4], F32, tag="s1")
        s2x = statp.tile([128, 4], F32, tag="s2")
        mux = statp.tile([128, 4], F32, tag="mu")
        rx = statp.tile([128, 4], F32, tag="r")
        tmpx = statp.tile([128, 4], F32, tag="tmp")
        for t in range(4):
            ln_stats(x_sb[:, t, :], s1x[:, t : t + 1], s2x[:, t : t + 1], dxA, dxB)
        ln_mu_r(s1x, s2x, mux, rx, tmpx, 4)
        for t in range(4):
            nc.vector.tensor_scalar(
                out=xn[:, t, :], in0=x_sb[:, t, :],
                scalar1=mux[:, t : t + 1], scalar2=rx[:, t : t + 1],
                op0=OP.subtract, op1=OP.mult,
            )
            nc.vector.tensor_tensor(
                out=xn[:, t, :], in0=xn[:, t, :], in1=reps[:, 0, :], op=OP.mult
            )
            nc.vector.tensor_tensor(
                out=xn[:, t, :], in0=xn[:, t, :], in1=reps[:, 1, :], op=OP.add
            )

        # xnT [c, n]
        xnT = sing.tile([128, 6, N], BF16)
        for ch in range(6):
            pst = ps_t.tile([128, N], BF16, tag="pst")
            for t in range(4):
                nc.tensor.transpose(
                    pst[:, t * 128 : (t + 1) * 128],
                    xn[:, t, ch * 128 : (ch + 1) * 128],
                    id128,
                )
            nc.scalar.copy(out=xnT[:, ch, :], in_=pst)

        # qkv natural [n, 3C]
        qkv = sing.tile([128, 4, 3 * C], BF16)
        OCH = [(0, 512), (512, 512), (1024, 512), (1536, 512), (2048, 256)]
        for t in range(4):
            for occ, ocs in OCH:
                pmm = ps_mm.tile([128, N], F32, tag="mm")
                for ch in range(6):
                    nc.tensor.matmul(
                        pmm[:, 0:ocs],
                        lhsT=xnT[:, ch, t * 128 : (t + 1) * 128],
                        rhs=wqkvt[:, ch, occ : occ + ocs],
                        start=(ch == 0), stop=False,
                    )
                nc.tensor.matmul(
                    pmm[:, 0:ocs], lhsT=ones_col[:, 0:128],
                    rhs=bqkv[:, occ : occ + ocs], start=False, stop=True,
                )
                eng = nc.vector if (occ // 512) % 2 == 0 else nc.scalar
                if eng is nc.vector:
                    nc.vector.tensor_copy(out=qkv[:, t, occ : occ + ocs], in_=pmm[:, 0:ocs])
                else:
                    nc.scalar.copy(out=qkv[:, t, occ : occ + ocs], in_=pmm[:, 0:ocs])

        # q/k LN (in place on qkv)
        s1q = statp.tile([128, 8], F32, tag="s1")
        s2q = statp.tile([128, 8], F32, tag="s2")
        muq = statp.tile([128, 8], F32, tag="mu")
        rq = statp.tile([128, 8], F32, tag="r")
        tmpq = statp.tile([128, 8], F32, tag="tmp")
        dqA = dumpp.tile([128, C], BF16, tag="dump")
        dqB = dumpp.tile([128, C], BF16, tag="dump2")
        for t in range(4):
            for qi, off in enumerate((0, C)):
                col = t * 2 + qi
                ln_stats(
                    qkv[:, t, off : off + C],
                    s1q[:, col : col + 1], s2q[:, col : col + 1], dqA, dqB,
                )
        ln_mu_r(s1q, s2q, muq, rq, tmpq, 8)
        for t in range(4):
            for qi, off in enumerate((0, C)):
                col = t * 2 + qi
                gr = 2 + 2 * qi  # qln_g row 2, kln_g row 4
                nc.vector.tensor_scalar(
                    out=qkv[:, t, off : off + C], in0=qkv[:, t, off : off + C],
                    scalar1=muq[:, col : col + 1], scalar2=rq[:, col : col + 1],
                    op0=OP.subtract, op1=OP.mult,
                )
                nc.vector.tensor_tensor(
                    out=qkv[:, t, off : off + C], in0=qkv[:, t, off : off + C],
                    in1=reps[:, gr, :], op=OP.mult,
                )
                nc.vector.tensor_tensor(
                    out=qkv[:, t, off : off + C], in0=qkv[:, t, off : off + C],
                    in1=reps[:, gr + 1, :], op=OP.add,
                )

        # kT [c, n] for all n; qT [c, i] for own rows (0:64 after roll)
        kT = sing.tile([128, 6, N], BF16)
        for ch in range(6):
            pst = ps_t.tile([128, N], BF16, tag="pst")
            for t in range(4):
                nc.tensor.transpose(
                    pst[:, t * 128 : (t + 1) * 128],
                    qkv[:, t, C + ch * 128 : C + (ch + 1) * 128],
                    id128,
                )
            nc.scalar.copy(out=kT[:, ch, :], in_=pst)
        qT = sing.tile([128, 6, IB], BF16)
        pst = ps_t.tile([128, N], BF16, tag="pst")
        for ch in range(6):
            nc.tensor.transpose(
                pst[:, ch * IB : (ch + 1) * IB],
                qkv[0:IB, 0, ch * 128 : (ch + 1) * 128],
                id128[0:IB, 0:IB],
            )
        nc.vector.tensor_copy(out=qT.rearrange("p a b -> p (a b)"), in_=pst[:, 0 : 6 * IB])

        # ================= pair path =================
        pv = pair_d.rearrange("(g t p) c -> g p t c", t=8, p=128)
        NG = NIJ // (8 * 128)  # 32 groups
        for g in range(NG):
            grp = pairp.tile([128, 8, C], BF16, tag="grp")
            nc.gpsimd.dma_start(out=grp, in_=pv[g])
            s1 = statp.tile([128, 8], F32, tag="s1")
            s2 = statp.tile([128, 8], F32, tag="s2")
            mu = statp.tile([128, 8], F32, tag="mu")
            r = statp.tile([128, 8], F32, tag="r")
            tmp = statp.tile([128, 8], F32, tag="tmp")
            dA = dumpp.tile([128, C], BF16, tag="dump")
            dB = dumpp.tile([128, C], BF16, tag="dump2")
            for t in range(8):
                ln_stats(grp[:, t, :], s1[:, t : t + 1], s2[:, t : t + 1], dA, dB)
            ln_mu_r(s1, s2, mu, r, tmp, 8)
            for t in range(8):
                nc.vector.tensor_scalar(
                    out=grp[:, t, :], in0=grp[:, t, :],
                    scalar1=mu[:, t : t + 1], scalar2=r[:, t : t + 1],
                    op0=OP.subtract, op1=OP.mult,
                )
            for q in range(2):
                pT = ptp.tile([128, 6, N], BF16, tag="pT")
                for ch in range(6):
                    pst2 = ps_t.tile([128, N], BF16, tag="pst")
                    for tt in range(4):
                        nc.tensor.transpose(
                            pst2[:, tt * 128 : (tt + 1) * 128],
                            grp[:, 4 * q + tt, ch * 128 : (ch + 1) * 128],
                            id128,
                        )
                    if ch % 2 == 0:
                        nc.scalar.copy(out=pT[:, ch, :], in_=pst2)
                    else:
                        nc.vector.tensor_copy(out=pT[:, ch, :], in_=pst2)
                bps = ps_mm.tile([128, N], F32, tag="mm")
                for ch in range(6):
                    nc.tensor.matmul(
                        bps[0:H, :], lhsT=wg[:, ch, :], rhs=pT[:, ch, :],
                        start=(ch == 0), stop=False,
                    )
                nc.tensor.matmul(
                    bps[0:H, :], lhsT=sbrow, rhs=ones_row, start=False, stop=True
                )
                stg = stagep.tile([H, N], BF16)
                nc.vector.tensor_copy(out=stg, in_=bps[0:H, :])
                pnl = 2 * g + q
                nc.sync.dma_start(
                    out=biasd[:, pnl * N : (pnl + 1) * N], in_=stg
                )

        # bias reshaped [i, h, j]
        bias_r = sing.tile([IB, H, N], BF16)
        bdr = biasd.rearrange("h (i j) -> h i j", i=IB)
        for h in range(H):
            nc.sync.dma_start(out=bias_r[:, h, :], in_=bdr[h])

        # ================= attention =================
        o_sb = sing.tile([IB, H, HD], BF16)
        for h in range(H):
            sps = ps_mm.tile([128, N], F32, tag="mm")
            bp = (h % 2) * 64
            nc.tensor.matmul(
                sps[0:IB, :],
                lhsT=qT[bp : bp + 64, h // 2, :],
                rhs=kT[bp : bp + 64, h // 2, :],
                start=True, stop=True,
            )
            sim = attnp.tile([IB, N], F32, tag="sim")
            nc.vector.scalar_tensor_tensor(
                out=sim, in0=sps[0:IB, :], scalar=0.125, in1=bias_r[:, h, :],
                op0=OP.mult, op1=OP.add,
            )
            mx = attnp.tile([IB, 1], F32, tag="mx")
            nc.vector.reduce_max(out=mx, in_=sim, axis=mybir.AxisListType.X)
            nc.vector.tensor_scalar(
                out=mx, in0=mx, scalar1=-1.0, scalar2=None, op0=OP.mult
            )
            esim = attnp.tile([IB, N], F32, tag="esim")
            den = attnp.tile([IB, 1], F32, tag="den")
            nc.scalar.activation(
                out=esim, in_=sim, func=AF.Exp, bias=mx, accum_out=den
            )
            nc.vector.reciprocal(out=den, in_=den)
            attn = attnp.tile([IB, N], BF16, tag="attn")
            nc.vector.tensor_scalar(
                out=attn, in0=esim, scalar1=den, scalar2=None, op0=OP.mult
            )
            aps = ps_t.tile([128, N], BF16, tag="pst")
            for jc in range(4):
                nc.tensor.transpose(
                    aps[:, jc * IB : (jc + 1) * IB],
                    attn[:, jc * 128 : (jc + 1) * 128],
                    id128[0:IB, 0:IB],
                )
            aT = attnp.tile([128, 4, IB], BF16, tag="aT")
            nc.vector.tensor_copy(
                out=aT.rearrange("p a b -> p (a b)"), in_=aps[:, 0 : 4 * IB]
            )
            ops = ps_mm.tile([128, N], F32, tag="mm")
            for jc in range(4):
                nc.tensor.matmul(
                    ops[0:IB, 0:HD],
                    lhsT=aT[:, jc, :],
                    rhs=qkv[:, jc, 2 * C + h * HD : 2 * C + (h + 1) * HD],
                    start=(jc == 0), stop=(jc == 3),
                )
            nc.scalar.copy(out=o_sb[:, h, :], in_=ops[0:IB, 0:HD])

        # ================= output proj =================
        o_fl = o_sb.rearrange("p a b -> p (a b)")
        oT = sing.tile([128, 6, IB], BF16)
        pso = ps_t.tile([128, N], BF16, tag="pst")
        for ch in range(6):
            nc.tensor.transpose(
                pso[:, ch * IB : (ch + 1) * IB],
                o_fl[:, ch * 128 : (ch + 1) * 128],
                id128[0:IB, 0:IB],
            )
        nc.vector.tensor_copy(out=oT.rearrange("p a b -> p (a b)"), in_=pso[:, 0 : 6 * IB])
        out_sb = sing.tile([IB, C], F32)
        for occ, ocs in [(0, 512), (512, 256)]:
            pps = ps_mm.tile([128, N], F32, tag="mm")
            for ch in range(6):
                nc.tensor.matmul(
                    pps[0:IB, 0:ocs],
                    lhsT=oT[:, ch, :],
                    rhs=wprojt[:, ch, occ : occ + ocs],
                    start=(ch == 0), stop=False,
                )
            nc.tensor.matmul(
                pps[0:IB, 0:ocs], lhsT=ones_col[:, 0:IB],
                rhs=bproj[:, occ : occ + ocs], start=False, stop=True,
            )
            nc.vector.tensor_copy(out=out_sb[:, occ : occ + ocs], in_=pps[0:IB, 0:ocs])
        nc.sync.dma_start(out=out_d, in_=out_sb)

    nc.compile()
    return nc


_NC = None
_LAST_MAPS = None


def kernel(x, pair, ln_g, ln_b, w_qkv, b_qkv, w_proj, b_proj, w_bias,
           pn_g, pn_b, qln_g, qln_b, kln_g, kln_b):
    global _NC
    x = np.asarray(x, np.float32)
    pair = np.asarray(pair, np.float32)
    b, n, _ = x.shape
    assert (b, n) == (1, N)

    if _NC is None:
        _NC = _build()

    wqkvt = np.ascontiguousarray(np.asarray(w_qkv, np.float32).T).astype(bf)
    wprojt = np.ascontiguousarray(np.asarray(w_proj, np.float32).T).astype(bf)
    wg = np.ascontiguousarray(
        (np.asarray(pn_g, np.float32)[:, None] * np.asarray(w_bias, np.float32).T)
    ).astype(bf)
    sbrow = (np.asarray(pn_b, np.float32) @ np.asarray(w_bias, np.float32).T)[
        None
    ].astype(bf)
    reps = np.stack(
        [np.asarray(a, np.float32) for a in (ln_g, ln_b, qln_g, qln_b, kln_g, kln_b)]
    ).astype(bf)
    bqkv = np.asarray(b_qkv, np.float32)[None].astype(bf)
    bproj = np.asarray(b_proj, np.float32)[None].astype(bf)

    in_maps = []
    for k in range(NCORES):
        ps = pair[0, k * IB : (k + 1) * IB]  # [64, 512, 768]
        ps = np.roll(ps, -k * IB, axis=1)  # roll j to match rolled x
        xk = np.roll(x[0], -k * IB, axis=0)
        in_maps.append(
            {
                "pair_s": np.ascontiguousarray(ps.reshape(NIJ, C), np.float32),
                "x_s": np.ascontiguousarray(xk, np.float32),
                "wqkvt": wqkvt,
                "bqkv": bqkv,
                "wprojt": wprojt,
                "bproj": bproj,
                "wg": wg,
                "sbrow": sbrow,
                "lnreps": reps,
            }
        )

    global _LAST_MAPS
    _LAST_MAPS = in_maps
    res = run_bass_kernel_spmd(_NC, in_maps, list(range(NCORES)))
    outs = [res.results[k]["out"] for k in range(NCORES)]
    return np.concatenate(outs, axis=0)[None].astype(np.float32)


# revision 11
# speedup vs baseline: 74.5903x; 74.5903x over previous
"""Trainium2 Bass kernel for nn_Attention_48687749268214.

Self-attention with pair-bias: LN(x) -> qkv -> q/k LN -> heads,
bias = einsum('bijc,hc->bhij', LN(pair), w_bias), softmax(qk/8+bias) @ v -> proj.

Sharding: sequence-shard the i axis across 8 cores (64 query rows each).
Each core gets its pair slice pair[i0:i0+64] (j-rolled so that the core's own
query rows sit at local rows 0:63 in its rolled copy of x), computes its own
64 output rows with no collectives; host concatenates.

Key math fold: bias = LN(pair) @ (pn_g * w_bias).T + pn_b @ w_bias.T, with the
LN applied in the natural [ij, c] layout (per-partition mean/rstd scalars),
then PE-transposed tiles feed a [12 x 512] matmul per i-row panel.
"""

import sys

sys.path.insert(0, "/opt/trn_rl_repo")

from contextlib import ExitStack

import ml_dtypes
import numpy as np

import concourse.bass as bass
import concourse.tile as tile
from concourse import bacc, mybir
from concourse.bass_utils import run_bass_kernel_spmd
from concourse.masks import make_identity

F32 = mybir.dt.float32
BF16 = mybir.dt.bfloat16
AF = mybir.ActivationFunctionType
OP = mybir.AluOpType

C = 768
H = 12
HD = 64
N = 512
NCORES = 8
IB = N // NCORES  # 64 i rows per core
NIJ = IB * N  # 32768 pair rows per core
EPS = 1e-5
RC = 1.0 / C

bf = ml_dtypes.bfloat16


def _build(repeat=1):
    nc = bacc.Bacc(
        "TRN2", target_bir_lowering=False, debug=False, num_devices=NCORES
    )

    pair_d = nc.dram_tensor("pair_s", [NIJ, C], F32, kind="ExternalInput").ap()
    x_d = nc.dram_tensor("x_s", [N, C], F32, kind="ExternalInput").ap()
    wqkvt_d = nc.dram_tensor("wqkvt", [C, 3 * C], BF16, kind="ExternalInput").ap()
    bqkv_d = nc.dram_tensor("bqkv", [1, 3 * C], BF16, kind="ExternalInput").ap()
    wprojt_d = nc.dram_tensor("wprojt", [C, C], BF16, kind="ExternalInput").ap()
    bproj_d = nc.dram_tensor("bproj", [1, C], BF16, kind="ExternalInput").ap()
    wg_d = nc.dram_tensor("wg", [C, H], BF16, kind="ExternalInput").ap()
    sbrow_d = nc.dram_tensor("sbrow", [1, H], BF16, kind="ExternalInput").ap()
    reps_d = nc.dram_tensor("lnreps", [6, C], BF16, kind="ExternalInput").ap()
    out_d = nc.dram_tensor("out", [IB, C], F32, kind="ExternalOutput").ap()
    biasd = nc.dram_tensor("bias_scratch", [H, NIJ], BF16).ap()

    with tile.TileContext(nc) as tc, ExitStack() as ctx:
        sing = ctx.enter_context(tc.tile_pool(name="sing", bufs=1))
        pairp = ctx.enter_context(tc.tile_pool(name="pairp", bufs=2))
        dumpp = ctx.enter_context(tc.tile_pool(name="dumpp", bufs=2))
        statp = ctx.enter_context(tc.tile_pool(name="statp", bufs=3))
        ptp = ctx.enter_context(tc.tile_pool(name="ptp", bufs=2))
        stagep = ctx.enter_context(tc.tile_pool(name="stagep", bufs=3))
        attnp = ctx.enter_context(tc.tile_pool(name="attnp", bufs=2))
        ps_t = ctx.enter_context(tc.tile_pool(name="ps_t", bufs=6, space="PSUM"))
        ps_mm = ctx.enter_context(tc.tile_pool(name="ps_mm", bufs=2, space="PSUM"))

        # ---- singles / weights ----
        id128 = sing.tile([128, 128], BF16)
        make_identity(nc, id128)
        ones_col = sing.tile([1, 128], BF16)
        nc.vector.memset(ones_col, 1.0)
        ones_row = sing.tile([1, N], BF16)
        nc.vector.memset(ones_row, 1.0)
        epsT = sing.tile([128, 1], F32)
        nc.vector.memset(epsT, EPS)

        wqkvt = sing.tile([128, 6, 3 * C], BF16)
        nc.sync.dma_start(out=wqkvt, in_=wqkvt_d.rearrange("(k p) o -> p k o", p=128))
        wprojt = sing.tile([128, 6, C], BF16)
        nc.sync.dma_start(out=wprojt, in_=wprojt_d.rearrange("(k p) o -> p k o", p=128))
        wg = sing.tile([128, 6, H], BF16)
        nc.sync.dma_start(out=wg, in_=wg_d.rearrange("(k p) o -> p k o", p=128))
        sbrow = sing.tile([1, H], BF16)
        nc.sync.dma_start(out=sbrow, in_=sbrow_d)
        bqkv = sing.tile([1, 3 * C], BF16)
        nc.sync.dma_start(out=bqkv, in_=bqkv_d)
        bproj = sing.tile([1, C], BF16)
        nc.sync.dma_start(out=bproj, in_=bproj_d)

        # replicated LN params: rows = ln_g, ln_b, qln_g, qln_b, kln_g, kln_b
        reps = sing.tile([128, 6, C], BF16)
        for rI in range(6):
            nc.gpsimd.dma_start(
                out=reps[:, rI, :],
                in_=bass.AP(
                    tensor=reps_d.tensor, offset=rI * C, ap=[[0, 128], [1, C]]
                ),
            )

        def ln_stats(src_ap, s1_ap, s2_ap, dA, dB):
            """accumulate sum (DVE) and sumsq (ACT) of a [p, C] tile."""
            nc.vector.tensor_scalar(
                out=dA, in0=src_ap, scalar1=1.0, scalar2=0.0, op0=OP.mult,
                op1=OP.add, accum_out=s1_ap,
            )
            nc.scalar.activation(out=dB, in_=src_ap, func=AF.Square, accum_out=s2_ap)

        def ln_mu_r(s1, s2, mu, r, tmp, w):
            """mu = s1/C ; r = rsqrt(s2/C - mu^2 + eps), all [p, w] f32."""
            nc.vector.tensor_scalar(
                out=mu[:, 0:w], in0=s1[:, 0:w], scalar1=RC, scalar2=None, op0=OP.mult
            )
            nc.vector.tensor_tensor(
                out=tmp[:, 0:w], in0=mu[:, 0:w], in1=mu[:, 0:w], op=OP.mult
            )
            nc.vector.scalar_tensor_tensor(
                out=tmp[:, 0:w], in0=s2[:, 0:w], scalar=RC, in1=tmp[:, 0:w],
                op0=OP.mult, op1=OP.subtract,
            )
            nc.scalar.activation(
                out=tmp[:, 0:w], in_=tmp[:, 0:w], func=AF.Ln, bias=epsT
            )
            nc.scalar.activation(out=r[:, 0:w], in_=tmp[:, 0:w], func=AF.Exp, scale=-0.5)

        # ================= compute body (repeatable for timing) =============
        from contextlib import nullcontext

        loop_cm = tc.For_i(0, repeat, 1) if repeat > 1 else nullcontext()
        with loop_cm:
            _compute(nc, tc, locals())

    nc.compile()
    return nc


def _compute(nc, tc, env):
    for _k, _v in env.items():
        globals()["_E_" + _k] = _v

    class _G:
        def __getattr__(self, k):
            return globals()["_E_" + k]

    g = _G()
    (sing, pairp, dumpp, statp, ptp, stagep, attnp, ps_t, ps_mm) = (
        g.sing, g.pairp, g.dumpp, g.statp, g.ptp, g.stagep, g.attnp, g.ps_t, g.ps_mm
    )
    id128, ones_col, ones_row, epsT = g.id128, g.ones_col, g.ones_row, g.epsT
    wqkvt, wprojt, wg, sbrow, bqkv, bproj, reps = (
        g.wqkvt, g.wprojt, g.wg, g.sbrow, g.bqkv, g.bproj, g.reps
    )
    ln_stats, ln_mu_r = g.ln_stats, g.ln_mu_r
    x_d, pair_d, out_d, biasd = g.x_d, g.pair_d, g.out_d, g.biasd

    if True:
        # ================= x path =================
        x_sb = sing.tile([128, 4, C], F32)
        nc.sync.dma_start(out=x_sb, in_=x_d.rearrange("(t p) c -> p t c", p=128))
        xn = sing.tile([128, 4, C], BF16)
        dxA = dumpp.tile([128, C], BF16, tag="dump")
        dxB = dumpp.tile([128, C], BF16, tag="dump2")
        s1x = statp.tile([128, 4], F32, tag="s1")
        s2x = statp.tile([128, 4], F32, tag="s2")
        mux = statp.tile([128, 4], F32, tag="mu")
        rx = statp.tile([128, 4], F32, tag="r")
        tmpx = statp.tile([128, 4], F32, tag="tmp")
        for t in range(4):
            ln_stats(x_sb[:, t, :], s1x[:, t : t + 1], s2x[:, t : t + 1], dxA, dxB)
        ln_mu_r(s1x, s2x, mux, rx, tmpx, 4)
        for t in range(4):
            nc.vector.tensor_scalar(
                out=xn[:, t, :], in0=x_sb[:, t, :],
                scalar1=mux[:, t : t + 1], scalar2=rx[:, t : t + 1],
                op0=OP.subtract, op1=OP.mult,
            )
            nc.vector.tensor_tensor(
                out=xn[:, t, :], in0=xn[:, t, :], in1=reps[:, 0, :], op=OP.mult
            )
            nc.vector.tensor_tensor(
                out=xn[:, t, :], in0=xn[:, t, :], in1=reps[:, 1, :], op=OP.add
            )

        # xnT [c, n]
        xnT = sing.tile([128, 6, N], BF16)
        for ch in range(6):
            pst = ps_t.tile([128, N], BF16, tag="pst")
            for t in range(4):
                nc.tensor.transpose(
                    pst[:, t * 128 : (t + 1) * 128],
                    xn[:, t, ch * 128 : (ch + 1) * 128],
                    id128,
                )
            nc.scalar.copy(out=xnT[:, ch, :], in_=pst)

        # qkv natural [n, 3C]
        qkv = sing.tile([128, 4, 3 * C], BF16)
        OCH = [(0, 512), (512, 512), (1024, 512), (1536, 512), (2048, 256)]
        for t in range(4):
            for occ, ocs in OCH:
                pmm = ps_mm.tile([128, N], F32, tag="mm")
                for ch in range(6):
                    nc.tensor.matmul(
                        pmm[:, 0:ocs],
                        lhsT=xnT[:, ch, t * 128 : (t + 1) * 128],
                        rhs=wqkvt[:, ch, occ : occ + ocs],
                        start=(ch == 0), stop=False,
                    )
                nc.tensor.matmul(
                    pmm[:, 0:ocs], lhsT=ones_col[:, 0:128],
                    rhs=bqkv[:, occ : occ + ocs], start=False, stop=True,
                )
                eng = nc.vector if (occ // 512) % 2 == 0 else nc.scalar
                if eng is nc.vector:
                    nc.vector.tensor_copy(out=qkv[:, t, occ : occ + ocs], in_=pmm[:, 0:ocs])
                else:
                    nc.scalar.copy(out=qkv[:, t, occ : occ + ocs], in_=pmm[:, 0:ocs])

        # q/k LN (in place on qkv)
        s1q = statp.tile([128, 8], F32, tag="s1")
        s2q = statp.tile([128, 8], F32, tag="s2")
        muq = statp.tile([128, 8], F32, tag="mu")
        rq = statp.tile([128, 8], F32, tag="r")
        tmpq = statp.tile([128, 8], F32, tag="tmp")
        dqA = dumpp.tile([128, C], BF16, tag="dump")
        dqB = dumpp.tile([128, C], BF16, tag="dump2")
        for t in range(4):
            for qi, off in enumerate((0, C)):
                col = t * 2 + qi
                ln_stats(
                    qkv[:, t, off : off + C],
                    s1q[:, col : col + 1], s2q[:, col : col + 1], dqA, dqB,
                )
        ln_mu_r(s1q, s2q, muq, rq, tmpq, 8)
        for t in range(4):
            for qi, off in enumerate((0, C)):
                col = t * 2 + qi
                gr = 2 + 2 * qi  # qln_g row 2, kln_g row 4
                nc.vector.tensor_scalar(
                    out=qkv[:, t, off : off + C], in0=qkv[:, t, off : off + C],
                    scalar1=muq[:, col : col + 1], scalar2=rq[:, col : col + 1],
                    op0=OP.subtract, op1=OP.mult,
                )
                nc.vector.tensor_tensor(
                    out=qkv[:, t, off : off + C], in0=qkv[:, t, off : off + C],
                    in1=reps[:, gr, :], op=OP.mult,
                )
                nc.vector.tensor_tensor(
                    out=qkv[:, t, off : off + C], in0=qkv[:, t, off : off + C],
                    in1=reps[:, gr + 1, :], op=OP.add,
                )

        # kT [c, n] for all n; qT [c, i] for own rows (0:64 after roll)
        kT = sing.tile([128, 6, N], BF16)
        for ch in range(6):
            pst = ps_t.tile([128, N], BF16, tag="pst")
            for t in range(4):
                nc.tensor.transpose(
                    pst[:, t * 128 : (t + 1) * 128],
                    qkv[:, t, C + ch * 128 : C + (ch + 1) * 128],
                    id128,
                )
            nc.scalar.copy(out=kT[:, ch, :], in_=pst)
        qT = sing.tile([128, 6, IB], BF16)
        pst = ps_t.tile([128, N], BF16, tag="pst")
        for ch in range(6):
            nc.tensor.transpose(
                pst[:, ch * IB : (ch + 1) * IB],
                qkv[0:IB, 0, ch * 128 : (ch + 1) * 128],
                id128[0:IB, 0:IB],
            )
        nc.vector.tensor_copy(out=qT.rearrange("p a b -> p (a b)"), in_=pst[:, 0 : 6 * IB])

        # ================= pair path =================
        pv = pair_d.rearrange("(g t p) c -> g p t c", t=8, p=128)
        NG = NIJ // (8 * 128)  # 32 groups
        for g in range(NG):
            grp = pairp.tile([128, 8, C], BF16, tag="grp")
            nc.gpsimd.dma_start(out=grp, in_=pv[g])
            s1 = statp.tile([128, 8], F32, tag="s1")
            s2 = statp.tile([128, 8], F32, tag="s2")
            mu = statp.tile([128, 8], F32, tag="mu")
            r = statp.tile([128, 8], F32, tag="r")
            tmp = statp.tile([128, 8], F32, tag="tmp")
            dA = dumpp.tile([128, C], BF16, tag="dump")
            dB = dumpp.tile([128, C], BF16, tag="dump2")
            for t in range(8):
                ln_stats(grp[:, t, :], s1[:, t : t + 1], s2[:, t : t + 1], dA, dB)
            ln_mu_r(s1, s2, mu, r, tmp, 8)
            for t in range(8):
                nc.vector.tensor_scalar(
                    out=grp[:, t, :], in0=grp[:, t, :],
                    scalar1=mu[:, t : t + 1], scalar2=r[:, t : t + 1],
                    op0=OP.subtract, op1=OP.mult,
                )
            for q in range(2):
                pT = ptp.tile([128, 6, N], BF16, tag="pT")
                for ch in range(6):
                    pst2 = ps_t.tile([128, N], BF16, tag="pst")
                    for tt in range(4):
                        nc.tensor.transpose(
                            pst2[:, tt * 128 : (tt + 1) * 128],
                            grp[:, 4 * q + tt, ch * 128 : (ch + 1) * 128],
                            id128,
                        )
                    if ch % 2 == 0:
                        nc.scalar.copy(out=pT[:, ch, :], in_=pst2)
                    else:
                        nc.vector.tensor_copy(out=pT[:, ch, :], in_=pst2)
                bps = ps_mm.tile([128, N], F32, tag="mm")
                for ch in range(6):
                    nc.tensor.matmul(
                        bps[0:H, :], lhsT=wg[:, ch, :], rhs=pT[:, ch, :],
                        start=(ch == 0), stop=False,
                    )
                nc.tensor.matmul(
                    bps[0:H, :], lhsT=sbrow, rhs=ones_row, start=False, stop=True
                )
                stg = stagep.tile([H, N], BF16)
                nc.vector.tensor_copy(out=stg, in_=bps[0:H, :])
                pnl = 2 * g + q
                nc.sync.dma_start(
                    out=biasd[:, pnl * N : (pnl + 1) * N], in_=stg
                )

        # bias reshaped [i, h, j]
        bias_r = sing.tile([IB, H, N], BF16)
        bdr = biasd.rearrange("h (i j) -> h i j", i=IB)
        for h in range(H):
            nc.sync.dma_start(out=bias_r[:, h, :], in_=bdr[h])

        # ================= attention =================
        o_sb = sing.tile([IB, H, HD], BF16)
        for h in range(H):
            sps = ps_mm.tile([128, N], F32, tag="mm")
            bp = (h % 2) * 64
            nc.tensor.matmul(
                sps[0:IB, :],
                lhsT=qT[bp : bp + 64, h // 2, :],
                rhs=kT[bp : bp + 64, h // 2, :],
                start=True, stop=True,
            )
            sim = attnp.tile([IB, N], F32, tag="sim")
            nc.vector.scalar_tensor_tensor(
                out=sim, in0=sps[0:IB, :], scalar=0.125, in1=bias_r[:, h, :],
                op0=OP.mult, op1=OP.add,
            )
            mx = attnp.tile([IB, 1], F32, tag="mx")
            nc.vector.reduce_max(out=mx, in_=sim, axis=mybir.AxisListType.X)
            nc.vector.tensor_scalar(
                out=mx, in0=mx, scalar1=-1.0, scalar2=None, op0=OP.mult
            )
            esim = attnp.tile([IB, N], F32, tag="esim")
            den = attnp.tile([IB, 1], F32, tag="den")
            nc.scalar.activation(
                out=esim, in_=sim, func=AF.Exp, bias=mx, accum_out=den
            )
            nc.vector.reciprocal(out=den, in_=den)
            attn = attnp.tile([IB, N], BF16, tag="attn")
            nc.vector.tensor_scalar(
                out=attn, in0=esim, scalar1=den, scalar2=None, op0=OP.mult
            )
            aps = ps_t.tile([128, N], BF16, tag="pst")
            for jc in range(4):
                nc.tensor.transpose(
                    aps[:, jc * IB : (jc + 1) * IB],
                    attn[:, jc * 128 : (jc + 1) * 128],
                    id128[0:IB, 0:IB],
                )
            aT = attnp.tile([128, 4, IB], BF16, tag="aT")
            nc.vector.tensor_copy(
                out=aT.rearrange("p a b -> p (a b)"), in_=aps[:, 0 : 4 * IB]
            )
            ops = ps_mm.tile([128, N], F32, tag="mm")
            for jc in range(4):
                nc.tensor.matmul(
                    ops[0:IB, 0:HD],
                    lhsT=aT[:, jc, :],
                    rhs=qkv[:, jc, 2 * C + h * HD : 2 * C + (h + 1) * HD],
                    start=(jc == 0), stop=(jc == 3),
                )
            nc.scalar.copy(out=o_sb[:, h, :], in_=ops[0:IB, 0:HD])

        # ================= output proj =================
        o_fl = o_sb.rearrange("p a b -> p (a b)")
        oT = sing.tile([128, 6, IB], BF16)
        pso = ps_t.tile([128, N], BF16, tag="pst")
        for ch in range(6):
            nc.tensor.transpose(
                pso[:, ch * IB : (ch + 1) * IB],
                o_fl[:, ch * 128 : (ch + 1) * 128],
                id128[0:IB, 0:IB],
            )
        nc.vector.tensor_copy(out=oT.rearrange("p a b -> p (a b)"), in_=pso[:, 0 : 6 * IB])
        out_sb = sing.tile([IB, C], F32)
        for occ, ocs in [(0, 512), (512, 256)]:
            pps = ps_mm.tile([128, N], F32, tag="mm")
            for ch in range(6):
                nc.tensor.matmul(
                    pps[0:IB, 0:ocs],
                    lhsT=oT[:, ch, :],
                    rhs=wprojt[:, ch, occ : occ + ocs],
                    start=(ch == 0), stop=False,
                )
            nc.tensor.matmul(
                pps[0:IB, 0:ocs], lhsT=ones_col[:, 0:IB],
                rhs=bproj[:, occ : occ + ocs], start=False, stop=True,
            )
            nc.vector.tensor_copy(out=out_sb[:, occ : occ + ocs], in_=pps[0:IB, 0:ocs])
        nc.sync.dma_start(out=out_d, in_=out_sb)


_NC = None
_LAST_MAPS = None


def prep_maps(x, pair, ln_g, ln_b, w_qkv, b_qkv, w_proj, b_proj, w_bias,
              pn_g, pn_b, qln_g, qln_b, kln_g, kln_b):
    x = np.asarray(x, np.float32)
    pair = np.asarray(pair, np.float32)
    wqkvt = np.ascontiguousarray(np.asarray(w_qkv, np.float32).T).astype(bf)
    wprojt = np.ascontiguousarray(np.asarray(w_proj, np.float32).T).astype(bf)
    wg = np.ascontiguousarray(
        (np.asarray(pn_g, np.float32)[:, None] * np.asarray(w_bias, np.float32).T)
    ).astype(bf)
    sbrow = (np.asarray(pn_b, np.float32) @ np.asarray(w_bias, np.float32).T)[
        None
    ].astype(bf)
    reps = np.stack(
        [np.asarray(a, np.float32) for a in (ln_g, ln_b, qln_g, qln_b, kln_g, kln_b)]
    ).astype(bf)
    bqkv = np.asarray(b_qkv, np.float32)[None].astype(bf)
    bproj = np.asarray(b_proj, np.float32)[None].astype(bf)

    in_maps = []
    for k in range(NCORES):
        ps = pair[0, k * IB : (k + 1) * IB]  # [64, 512, 768]
        ps = np.roll(ps, -k * IB, axis=1)  # roll j to match rolled x
        xk = np.roll(x[0], -k * IB, axis=0)
        in_maps.append(
            {
                "pair_s": np.ascontiguousarray(ps.reshape(NIJ, C), np.float32),
                "x_s": np.ascontiguousarray(xk, np.float32),
                "wqkvt": wqkvt,
                "bqkv": bqkv,
                "wprojt": wprojt,
                "bproj": bproj,
                "wg": wg,
                "sbrow": sbrow,
                "lnreps": reps,
            }
        )

    return in_maps


def kernel(**inputs):
    global _NC, _LAST_MAPS
    if _NC is None:
        _NC = _build()
    in_maps = prep_maps(**inputs)
    _LAST_MAPS = in_maps
    res = run_bass_kernel_spmd(_NC, in_maps, list(range(NCORES)))
    outs = [res.results[k]["out"] for k in range(NCORES)]
    return np.concatenate(outs, axis=0)[None].astype(np.float32)
